# revision 9
# baseline (speedup 1.0000x reference)
"""FISTA sparse-coding encoder kernel for Trainium2 (8 NeuronCores).

Problem: x [2,10,20480] f32, Drr/Dtheta [40] f32.
  D = normalized dictionary [10, 161]
  A = I - D^T D / L,  DtY = D^T Y / L,  lam = gamma / L
  40 FISTA iterations: xn = softshrink(A @ y + DtY); y = xn + m (xn - x_old)
  output sparsecode [2, 161, 20480].

Design "v5": data-parallel over columns (5120 per core), u-form momentum
  u_i = A x_i + DtY,  xn_{i+1} = shrink((1+m) u_i - m u_{i-1}, lam),
with a COLUMN-PACKED TAIL STATE.

State layout per core (columns split in halves blk0 = [0:2560),
blk1 = [2560:5120)):
  xH  [128, 5120] f32r : rows 0:128 of x.
  xTP [86, 2560]  f32r : [Y0(10); xT0(33); xT1(33); Y1(10)] - the 33 tail
        rows of x for each half, column-packed, with the constant Y rows
        glued on the outside so every matmul sees a contiguous partition
        range:
          head-out mm (blk0): rhs = xTP[0:43]  = [Y0; xT0]
          head-out mm (blk1): rhs = xTP[43:86] = [xT1; Y1]
          tail-out mm:        rhs = xTP[0:86]  (block-diag weights)
          tail shrink writes  xTP[10:76] (xT0; xT1 contiguous).

Per iteration (5120 cols):
  PE  : 3.5 passes = head-out 2 x 5120 + tail-out 1.5 x 5120 free cycles
        (the packed tail psum [66, 512] covers TWO column halves per pass;
        xH contributions enter via [128, 66] half-zero weights).
  ACT : 3 of 5 head-group PSUM evacuations (plain copies).
  DMA : 2 of 5 head evacuations + all 5 tail evacuations (partition-shift
        psum[0:66] -> uevT[10:76]), on the SP queue.
  DVE : fused shrink+momentum custom op only (8.2 us/iter) - the wall.
No repack DMAs: the shrink writes the packed tail state in place.
"""

import numpy as np

# ---------------------------------------------------------------- constants
B, T, N_POLES, P = 2, 10, 40, 20480
MAX_ITER = 40
GAMMA = 0.01
K = 4 * N_POLES + 1          # 161
NCORES = 8
NCOLS = B * P // NCORES      # 5120 columns per core
HALF = NCOLS // 2            # 2560
BLK = 512                    # matmul free dim (one PSUM bank)
GRP = 1024                   # head PSUM group (2 banks)
NGRP = NCOLS // GRP          # 5
NTC = HALF // BLK            # 5 packed tail chunks
KH = 128                     # head rows
KT = K - KH                  # 33 tail rows
KTP = 86                     # packed-tail partitions (x0,x1,Y0,Y1)

_cache = {}


# ------------------------------------------------------------- custom DVE op
def _register_shrinkmom3():
    """out = w - clip(w, -C2, C2)  with  w = in0*s0 + in1*s1.

    Equals softshrink(w, C2); both momentum scales live in the op.
    """
    import concourse.dve_ops as dve_ops
    from concourse.dve_spec import Spec, Src0, Src1, C2, Zero, maxx, minn, lower
    from concourse.dve_spec import _has_src1 as has_src1
    from concourse.dve_spec import C0, C1
    from concourse.dve_uop import DveOpSpec

    name = "ANT_SHRINKMOM3_FISTA"
    if any(op.name == name for op in dve_ops.OPS):
        return next(op for op in dve_ops.OPS if op.name == name)

    w = Src0 * C0 + Src1 * C1
    spec = Spec(
        body=w - minn(maxx(w, Zero - C2), C2),
        reference=lambda in0, in1, s0=1.0, s1=0.0, imm2=0.0: (
            lambda ww: (ww - np.minimum(np.maximum(ww, -imm2), imm2))
            .astype(np.float32)
        )(in0 * s0 + in1 * s1),
    )
    op = dve_ops.DveOp(name, spec, subdim=False, uops_sha={})
    dve_ops.OPS.append(op)
    dve_ops.CUSTOM_DVE_SPECS[name] = spec
    dve_ops._SUB_OPCODE_FOR_NAME[name] = (
        dve_ops._CUSTOM_DVE_ROW_BASE + len(dve_ops.OPS) - 1
    )
    for ver in ("v3", "v4"):
        compiled = DveOpSpec(
            name=name,
            opcode=dve_ops.get_dve_sub_opcode(name),
            uops=lower(spec, ver=ver),
            rd1_en=has_src1(spec),
        )
        op.uops_sha[ver] = compiled.sha(ver)
    return op


# ------------------------------------------------------------ host constants
def _host_constants(Drr, Dtheta):
    r = Drr.astype(np.float64)
    th = Dtheta.astype(np.float64)
    i = np.arange(T, dtype=np.float64)[:, None]
    pr = r[None, :] ** i
    sgn = np.where(np.arange(T)[:, None] % 2 == 0, 1.0, -1.0)
    c = np.cos(i * th[None, :])
    s = np.sin(i * th[None, :])
    ones = np.ones((T, 1))
    dic = np.concatenate([ones, pr * c, sgn * pr * c, pr * s, sgn * pr * s], axis=1)
    G = np.linalg.norm(dic, axis=0)
    G = np.where(G == 0, np.sqrt(float(T)), G)
    D = (dic / G)                               # [T, K] f64

    DtD = D.T @ D
    L = float(np.linalg.norm(DtD))              # Frobenius
    A = np.eye(K) - DtD / L                     # [K, K] symmetric
    lam = float(GAMMA / L)
    DoL = D / L                                 # [T, K]

    # lhsT convention: out[m, c] = sum_p lhsT[p, m] rhs[p, c]
    # Packed-tail tile xTP rows: [x0(0:33); x1(33:66); Y0(66:76);
    # Y1(76:86)].  Every engine access starts at base partition 0 (or 64
    # for the small init copies); zero weight rows cover the other block.
    f32 = np.float32
    wh1 = A[0:KH, 0:KH].astype(f32)                       # [128,128] x_head
    a_th = A[KH:K, 0:KH]                                  # [33, 128]
    d_h = DoL[:, 0:KH]                                    # [10, 128]
    z33h = np.zeros((KT, KH))
    z10h = np.zeros((T, KH))
    # head-out blk0: rhs = xTP[0:76] (x0, x1, Y0)
    wh2a = np.concatenate([a_th, z33h, d_h], axis=0).astype(f32)   # [76,128]
    # head-out blk1: rhs = xTP[0:86] (x0, x1, Y0, Y1)
    wh2b = np.concatenate([z33h, a_th, z10h, d_h], axis=0).astype(f32)
    # tail-out (packed psum [66] = [uT0(33); uT1(33)]) from xH halves:
    a_ht = A[0:KH, KH:K]                                  # [128, 33]
    z = np.zeros((KH, KT))
    wtx0 = np.concatenate([a_ht, z], axis=1).astype(f32)      # [128, 66]
    wtx1 = np.concatenate([z, a_ht], axis=1).astype(f32)      # [128, 66]
    # tail-out self part, rhs = xTP[0:86]
    a_tt = A[KH:K, KH:K]                                  # [33, 33]
    d_t = DoL[:, KH:K]                                    # [10, 33]
    z33 = np.zeros((KT, KT))
    z10 = np.zeros((T, KT))
    wts = np.concatenate([
        np.concatenate([a_tt, z33], axis=1),              # x0   -> uT0
        np.concatenate([z33, a_tt], axis=1),              # x1   -> uT1
        np.concatenate([d_t, z10], axis=1),               # Y0   -> uT0
        np.concatenate([z10, d_t], axis=1),               # Y1   -> uT1
    ], axis=0).astype(f32)                                # [86, 66]

    # momentum coefficients m_i = (t_i - 1)/t_{i+1}, t_0 = 1
    ms = []
    t = 1.0
    for _ in range(MAX_ITER):
        t_new = (1.0 + np.sqrt(1.0 + 4.0 * t * t)) / 2.0
        ms.append((t - 1.0) / t_new)
        t = t_new
    wdict = {"wh1": wh1, "wh2a": wh2a, "wh2b": wh2b,
             "wtx0": wtx0, "wtx1": wtx1, "wts": wts}
    return wdict, lam, ms


# ------------------------------------------------------------- bass program
def _build_program():
    import concourse.mybir as mybir
    import concourse.tile as tile
    from concourse import bacc

    fused_op = _register_shrinkmom3()

    f32 = mybir.dt.float32
    f32r = mybir.dt.float32r

    nc = bacc.Bacc("TRN2", target_bir_lowering=False, debug=False,
                   num_devices=NCORES)

    ycols = nc.dram_tensor("ycols", [T, NCOLS], f32, kind="ExternalInput")
    wshapes = {"wh1": [KH, KH], "wh2a": [76, KH], "wh2b": [KTP, KH],
               "wtx0": [KH, 2 * KT], "wtx1": [KH, 2 * KT], "wts": [KTP, 2 * KT]}
    d_w = {nm: nc.dram_tensor(nm, shp, f32, kind="ExternalInput")
           for nm, shp in wshapes.items()}
    out = nc.dram_tensor("out", [K, NCOLS], f32, kind="ExternalOutput")

    lam, ms = _cache["consts_meta"]

    with tile.TileContext(nc) as tc:
        with (
            tc.tile_pool(name="state", bufs=1) as st,
            tc.tile_pool(name="wts", bufs=1) as wp,
            tc.tile_pool(name="psH", bufs=2, space="PSUM") as psHp,
            tc.tile_pool(name="psT", bufs=2, space="PSUM") as psTp,
        ):
            # ---- persistent state -------------------------------------
            xH = [st.tile([KH, NCOLS], f32r, tag=f"xH{b}", name=f"xH{b}")
                  for b in range(2)]
            xTP = [st.tile([KTP, HALF], f32r, tag=f"xTP{b}", name=f"xTP{b}")
                   for b in range(2)]
            uevH = [st.tile([KH, NCOLS], f32, tag=f"uevH{b}", name=f"uevH{b}")
                    for b in range(2)]
            uevT = [st.tile([2 * KT, HALF], f32, tag=f"uevT{b}",
                            name=f"uevT{b}") for b in range(2)]

            # fp32 staging for DMA'd weights -> rounded f32r copies
            wt_st = {}
            wt_r = {}
            for nm, shp in wshapes.items():
                wt_st[nm] = wp.tile(shp, f32, tag=f"st_{nm}", name=f"st_{nm}")
                wt_r[nm] = wp.tile(shp, f32r, tag=f"r_{nm}", name=f"r_{nm}")
            for nm in ("wts", "wh2a", "wh2b", "wtx0", "wtx1", "wh1"):
                nc.sync.dma_start(wt_st[nm][:], d_w[nm][:])
            engs = [nc.scalar, nc.vector, nc.gpsimd]
            for j, nm in enumerate(wshapes):
                e = engs[j % 3]
                if e is nc.scalar:
                    e.copy(wt_r[nm][:], wt_st[nm][:])
                else:
                    e.tensor_copy(wt_r[nm][:], wt_st[nm][:])

            # ---- init: one staging tile shaped like xTP: rows 0:66 = 0
            # (the x rows; iter-0 matmuls read them with zero weights, the
            # data must still be finite), 66:76 = Y0, 76:86 = Y1.
            with tc.tile_pool(name="init", bufs=1) as ip:
                zst = ip.tile([KTP, HALF], f32, tag="zst", name="zst")
                nc.gpsimd.memset(zst[0:66, :], 0.0)
                nc.sync.dma_start(zst[66:76, :], ycols[:, 0:HALF])
                nc.scalar.dma_start(zst[76:86, :], ycols[:, HALF:NCOLS])
                for cc in (1, 0):
                    ph = slice(cc * (HALF // 2), (cc + 1) * (HALF // 2))
                    nc.gpsimd.tensor_copy(xTP[0][0:64, ph], zst[0:64, ph])
                    nc.scalar.copy(xTP[0][64:KTP, ph], zst[64:KTP, ph])
                    nc.scalar.copy(xTP[1][64:KTP, ph], zst[64:KTP, ph])

            def mm(ps, lhsT, rhs, start, stop):
                nc.tensor.matmul(ps, lhsT, rhs, start=start, stop=stop)

            # Unit schedule: H u = head source cols [1024u, 1024u+1024);
            # T k = packed tail cols [1024k, min(1024k+1024, 2560)).
            # Order chosen so each unit's cross-iteration inputs (previous
            # iteration's shrink outputs) are produced well before use.
            ORDER = [("T", 2), ("H", 0), ("H", 2), ("H", 4),
                     ("T", 0), ("H", 1), ("H", 3), ("T", 1)]

            f32c = mybir.dt.float32
            for it in range(MAX_ITER):
                cur = it % 2
                nxt = (it + 1) % 2           # also the "previous" uev buffer
                m_prev = ms[it - 1] if it > 0 else 0.0
                s0 = float(1.0 + m_prev)
                s1 = float(-m_prev)
                last = it == MAX_ITER - 1
                # At it=0 s1=0 and the previous-u buffer is uninitialized:
                # alias in1 to the current buffer (contributes s1*in1 = 0).
                hp = cur if it == 0 else nxt

                for kind, u in ORDER:
                    if kind == "H":
                        gs = slice(u * GRP, (u + 1) * GRP)
                        wh = psHp.tile([KH, GRP], f32c, tag="wh", name="wh")
                        for bq in range(GRP // BLK):
                            c0 = u * GRP + bq * BLK
                            pb = slice(bq * BLK, (bq + 1) * BLK)
                            if c0 < HALF:
                                rhs2 = xTP[cur][0:76, c0:c0 + BLK]
                                w2 = wt_r["wh2a"][:]
                            else:
                                rhs2 = xTP[cur][0:KTP,
                                                c0 - HALF:c0 - HALF + BLK]
                                w2 = wt_r["wh2b"][:]
                            if it == 0:
                                mm(wh[:, pb], w2, rhs2, True, True)
                            else:
                                mm(wh[:, pb], wt_r["wh1"][:],
                                   xH[cur][:, c0:c0 + BLK], True, False)
                                mm(wh[:, pb], w2, rhs2, False, True)
                        if not last:
                            nc.scalar.copy(uevH[cur][:, gs], wh[:])
                            nc.vector._custom_dve(
                                fused_op, out=xH[nxt][:, gs],
                                in0=uevH[cur][:, gs], in1=uevH[hp][:, gs],
                                s0=s0, s1=s1, imm2=float(lam))
                        else:
                            # final iteration: shrink straight from PSUM and
                            # stream the head output out.
                            nc.vector._custom_dve(
                                fused_op, out=xH[nxt][:, gs],
                                in0=wh[:], in1=uevH[hp][:, gs],
                                s0=s0, s1=s1, imm2=float(lam))
                            nc.gpsimd.dma_start(out[0:KH, gs],
                                                xH[nxt][:, gs].bitcast(f32c))
                    else:
                        lo = u * GRP
                        hi = min(lo + GRP, HALF)
                        w = hi - lo
                        wt = psTp.tile([2 * KT, GRP], f32c, tag="wt",
                                       name="wt")
                        for bq in range(w // BLK):
                            pc0 = lo + bq * BLK
                            pb = slice(bq * BLK, (bq + 1) * BLK)
                            pcs = slice(pc0, pc0 + BLK)
                            if it == 0:
                                mm(wt[:, pb], wt_r["wts"][:],
                                   xTP[cur][:, pcs], True, True)
                            else:
                                mm(wt[:, pb], wt_r["wts"][:],
                                   xTP[cur][:, pcs], True, False)
                                mm(wt[:, pb], wt_r["wtx0"][:],
                                   xH[cur][:, pcs], False, False)
                                mm(wt[:, pb], wt_r["wtx1"][:],
                                   xH[cur][:, HALF + pc0:HALF + pc0 + BLK],
                                   False, True)
                        nc.scalar.copy(uevT[cur][:, lo:hi], wt[:, 0:w])
                        nc.vector._custom_dve(
                            fused_op, out=xTP[nxt][0:2 * KT, lo:hi],
                            in0=uevT[cur][:, lo:hi], in1=uevT[hp][:, lo:hi],
                            s0=s0, s1=s1, imm2=float(lam))
                        if last:
                            nc.sync.dma_start(
                                out[KH:K, lo:hi],
                                xTP[nxt][0:KT, lo:hi].bitcast(f32c))
                            nc.sync.dma_start(
                                out[KH:K, HALF + lo:HALF + hi],
                                xTP[nxt][KT:2 * KT, lo:hi].bitcast(f32c))
    nc.finalize()
    return nc


def _get_program(lam, ms):
    key = (round(lam, 12), tuple(round(m, 9) for m in ms))
    if _cache.get("key") != key:
        _cache["consts_meta"] = (lam, ms)
        _cache["nc"] = _build_program()
        _cache["key"] = key
    return _cache["nc"]


# ------------------------------------------------------------------- kernel
def kernel(x, Drr, Dtheta):
    from concourse.bass_utils import run_bass_kernel_spmd

    wdict, lam, ms = _host_constants(Drr, Dtheta)
    nc = _get_program(lam, ms)

    xc = np.ascontiguousarray(
        np.transpose(x.astype(np.float32), (1, 0, 2)).reshape(T, B * P))

    in_maps = []
    for c in range(NCORES):
        m = {"ycols": np.ascontiguousarray(xc[:, c * NCOLS:(c + 1) * NCOLS])}
        m.update({k: np.ascontiguousarray(v) for k, v in wdict.items()})
        in_maps.append(m)

    res = run_bass_kernel_spmd(nc, in_maps, core_ids=list(range(NCORES)))
    _cache["last_res"] = res
    full = np.concatenate([r["out"] for r in res.results], axis=1)  # [K, B*P]
    return np.ascontiguousarray(
        full.reshape(K, B, P).transpose(1, 0, 2)).astype(np.float32)


if __name__ == "__main__":
    x = np.random.randn(B, T, P).astype(np.float32)
    Drr = np.random.rand(N_POLES).astype(np.float32)
    Dtheta = np.random.rand(N_POLES).astype(np.float32)
    o = kernel(x, Drr, Dtheta)
    print(o.shape, o.dtype)


# revision 10
# speedup vs baseline: 1.0058x; 1.0058x over previous
"""FISTA sparse-coding encoder kernel for Trainium2 (8 NeuronCores).

Problem: x [2,10,20480] f32, Drr/Dtheta [40] f32.
  D = normalized dictionary [10, 161]
  A = I - D^T D / L,  DtY = D^T Y / L,  lam = gamma / L
  40 FISTA iterations: xn = softshrink(A @ y + DtY); y = xn + m (xn - x_old)
  output sparsecode [2, 161, 20480].

Design "v5": data-parallel over columns (5120 per core), u-form momentum
  u_i = A x_i + DtY,  xn_{i+1} = shrink((1+m) u_i - m u_{i-1}, lam),
with a COLUMN-PACKED TAIL STATE.

State layout per core (columns split in halves blk0 = [0:2560),
blk1 = [2560:5120)):
  xH  [128, 5120] f32r : rows 0:128 of x.
  xTP [86, 2560]  f32r : [Y0(10); xT0(33); xT1(33); Y1(10)] - the 33 tail
        rows of x for each half, column-packed, with the constant Y rows
        glued on the outside so every matmul sees a contiguous partition
        range:
          head-out mm (blk0): rhs = xTP[0:43]  = [Y0; xT0]
          head-out mm (blk1): rhs = xTP[43:86] = [xT1; Y1]
          tail-out mm:        rhs = xTP[0:86]  (block-diag weights)
          tail shrink writes  xTP[10:76] (xT0; xT1 contiguous).

Per iteration (5120 cols):
  PE  : 3.5 passes = head-out 2 x 5120 + tail-out 1.5 x 5120 free cycles
        (the packed tail psum [66, 512] covers TWO column halves per pass;
        xH contributions enter via [128, 66] half-zero weights).
  ACT : 3 of 5 head-group PSUM evacuations (plain copies).
  DMA : 2 of 5 head evacuations + all 5 tail evacuations (partition-shift
        psum[0:66] -> uevT[10:76]), on the SP queue.
  DVE : fused shrink+momentum custom op only (8.2 us/iter) - the wall.
No repack DMAs: the shrink writes the packed tail state in place.
"""

import numpy as np

# ---------------------------------------------------------------- constants
B, T, N_POLES, P = 2, 10, 40, 20480
MAX_ITER = 40
GAMMA = 0.01
K = 4 * N_POLES + 1          # 161
NCORES = 8
NCOLS = B * P // NCORES      # 5120 columns per core
HALF = NCOLS // 2            # 2560
BLK = 512                    # matmul free dim (one PSUM bank)
GRP = 1024                   # head PSUM group (2 banks)
NGRP = NCOLS // GRP          # 5
NTC = HALF // BLK            # 5 packed tail chunks
KH = 128                     # head rows
KT = K - KH                  # 33 tail rows
KTP = 86                     # packed-tail partitions (x0,x1,Y0,Y1)

_cache = {}


# ------------------------------------------------------------- custom DVE op
def _register_shrinkmom3():
    """out = w - clip(w, -C2, C2)  with  w = in0*s0 + in1*s1.

    Equals softshrink(w, C2); both momentum scales live in the op.
    """
    import concourse.dve_ops as dve_ops
    from concourse.dve_spec import Spec, Src0, Src1, C2, Zero, maxx, minn, lower
    from concourse.dve_spec import _has_src1 as has_src1
    from concourse.dve_spec import C0, C1
    from concourse.dve_uop import DveOpSpec

    name = "ANT_SHRINKMOM3_FISTA"
    if any(op.name == name for op in dve_ops.OPS):
        return next(op for op in dve_ops.OPS if op.name == name)

    w = Src0 * C0 + Src1 * C1
    spec = Spec(
        body=w - minn(maxx(w, Zero - C2), C2),
        reference=lambda in0, in1, s0=1.0, s1=0.0, imm2=0.0: (
            lambda ww: (ww - np.minimum(np.maximum(ww, -imm2), imm2))
            .astype(np.float32)
        )(in0 * s0 + in1 * s1),
    )
    op = dve_ops.DveOp(name, spec, subdim=False, uops_sha={})
    dve_ops.OPS.append(op)
    dve_ops.CUSTOM_DVE_SPECS[name] = spec
    dve_ops._SUB_OPCODE_FOR_NAME[name] = (
        dve_ops._CUSTOM_DVE_ROW_BASE + len(dve_ops.OPS) - 1
    )
    for ver in ("v3", "v4"):
        compiled = DveOpSpec(
            name=name,
            opcode=dve_ops.get_dve_sub_opcode(name),
            uops=lower(spec, ver=ver),
            rd1_en=has_src1(spec),
        )
        op.uops_sha[ver] = compiled.sha(ver)
    return op


# ------------------------------------------------------------ host constants
def _host_constants(Drr, Dtheta):
    r = Drr.astype(np.float64)
    th = Dtheta.astype(np.float64)
    i = np.arange(T, dtype=np.float64)[:, None]
    pr = r[None, :] ** i
    sgn = np.where(np.arange(T)[:, None] % 2 == 0, 1.0, -1.0)
    c = np.cos(i * th[None, :])
    s = np.sin(i * th[None, :])
    ones = np.ones((T, 1))
    dic = np.concatenate([ones, pr * c, sgn * pr * c, pr * s, sgn * pr * s], axis=1)
    G = np.linalg.norm(dic, axis=0)
    G = np.where(G == 0, np.sqrt(float(T)), G)
    D = (dic / G)                               # [T, K] f64

    DtD = D.T @ D
    L = float(np.linalg.norm(DtD))              # Frobenius
    A = np.eye(K) - DtD / L                     # [K, K] symmetric
    lam = float(GAMMA / L)
    DoL = D / L                                 # [T, K]

    # lhsT convention: out[m, c] = sum_p lhsT[p, m] rhs[p, c]
    # Packed-tail tile xTP rows: [x0(0:33); x1(33:66); Y0(66:76);
    # Y1(76:86)].  Every engine access starts at base partition 0 (or 64
    # for the small init copies); zero weight rows cover the other block.
    f32 = np.float32
    wh1 = A[0:KH, 0:KH].astype(f32)                       # [128,128] x_head
    a_th = A[KH:K, 0:KH]                                  # [33, 128]
    d_h = DoL[:, 0:KH]                                    # [10, 128]
    z33h = np.zeros((KT, KH))
    z10h = np.zeros((T, KH))
    # head-out blk0: rhs = xTP[0:76] (x0, x1, Y0)
    wh2a = np.concatenate([a_th, z33h, d_h], axis=0).astype(f32)   # [76,128]
    # head-out blk1: rhs = xTP[0:86] (x0, x1, Y0, Y1)
    wh2b = np.concatenate([z33h, a_th, z10h, d_h], axis=0).astype(f32)
    # tail-out (packed psum [66] = [uT0(33); uT1(33)]) from xH halves:
    a_ht = A[0:KH, KH:K]                                  # [128, 33]
    z = np.zeros((KH, KT))
    wtx0 = np.concatenate([a_ht, z], axis=1).astype(f32)      # [128, 66]
    wtx1 = np.concatenate([z, a_ht], axis=1).astype(f32)      # [128, 66]
    # tail-out self part, rhs = xTP[0:86]
    a_tt = A[KH:K, KH:K]                                  # [33, 33]
    d_t = DoL[:, KH:K]                                    # [10, 33]
    z33 = np.zeros((KT, KT))
    z10 = np.zeros((T, KT))
    wts = np.concatenate([
        np.concatenate([a_tt, z33], axis=1),              # x0   -> uT0
        np.concatenate([z33, a_tt], axis=1),              # x1   -> uT1
        np.concatenate([d_t, z10], axis=1),               # Y0   -> uT0
        np.concatenate([z10, d_t], axis=1),               # Y1   -> uT1
    ], axis=0).astype(f32)                                # [86, 66]

    # momentum coefficients m_i = (t_i - 1)/t_{i+1}, t_0 = 1
    ms = []
    t = 1.0
    for _ in range(MAX_ITER):
        t_new = (1.0 + np.sqrt(1.0 + 4.0 * t * t)) / 2.0
        ms.append((t - 1.0) / t_new)
        t = t_new
    wdict = {"wh1": wh1, "wh2a": wh2a, "wh2b": wh2b,
             "wtx0": wtx0, "wtx1": wtx1, "wts": wts}
    return wdict, lam, ms


# ------------------------------------------------------------- bass program
def _build_program():
    import concourse.mybir as mybir
    import concourse.tile as tile
    from concourse import bacc

    fused_op = _register_shrinkmom3()

    f32 = mybir.dt.float32
    f32r = mybir.dt.float32r

    nc = bacc.Bacc("TRN2", target_bir_lowering=False, debug=False,
                   num_devices=NCORES)

    ycols = nc.dram_tensor("ycols", [T, NCOLS], f32, kind="ExternalInput")
    wshapes = {"wh1": [KH, KH], "wh2a": [76, KH], "wh2b": [KTP, KH],
               "wtx0": [KH, 2 * KT], "wtx1": [KH, 2 * KT], "wts": [KTP, 2 * KT]}
    d_w = {nm: nc.dram_tensor(nm, shp, f32, kind="ExternalInput")
           for nm, shp in wshapes.items()}
    out = nc.dram_tensor("out", [K, NCOLS], f32, kind="ExternalOutput")

    lam, ms = _cache["consts_meta"]

    with tile.TileContext(nc) as tc:
        with (
            tc.tile_pool(name="state", bufs=1) as st,
            tc.tile_pool(name="wts", bufs=1) as wp,
            tc.tile_pool(name="psH", bufs=2, space="PSUM") as psHp,
            tc.tile_pool(name="psT", bufs=2, space="PSUM") as psTp,
        ):
            # ---- persistent state -------------------------------------
            xH = [st.tile([KH, NCOLS], f32r, tag=f"xH{b}", name=f"xH{b}")
                  for b in range(2)]
            xTP = [st.tile([KTP, HALF], f32r, tag=f"xTP{b}", name=f"xTP{b}")
                   for b in range(2)]
            uevH = [st.tile([KH, NCOLS], f32, tag=f"uevH{b}", name=f"uevH{b}")
                    for b in range(2)]
            uevT = [st.tile([2 * KT, HALF], f32, tag=f"uevT{b}",
                            name=f"uevT{b}") for b in range(2)]

            # fp32 staging for DMA'd weights -> rounded f32r copies
            wt_st = {}
            wt_r = {}
            for nm, shp in wshapes.items():
                wt_st[nm] = wp.tile(shp, f32, tag=f"st_{nm}", name=f"st_{nm}")
                wt_r[nm] = wp.tile(shp, f32r, tag=f"r_{nm}", name=f"r_{nm}")
            for nm in ("wts", "wh2a", "wh2b", "wtx0", "wtx1", "wh1"):
                nc.sync.dma_start(wt_st[nm][:], d_w[nm][:])
            engs = [nc.scalar, nc.vector, nc.gpsimd]
            for j, nm in enumerate(wshapes):
                e = engs[j % 3]
                if e is nc.scalar:
                    e.copy(wt_r[nm][:], wt_st[nm][:])
                else:
                    e.tensor_copy(wt_r[nm][:], wt_st[nm][:])

            # ---- init: one staging tile shaped like xTP: rows 0:66 = 0
            # (the x rows; iter-0 matmuls read them with zero weights, the
            # data must still be finite), 66:76 = Y0, 76:86 = Y1.
            with tc.tile_pool(name="init", bufs=1) as ip:
                zst = ip.tile([KTP, HALF], f32, tag="zst", name="zst")
                nc.gpsimd.memset(zst[0:66, :], 0.0)
                nc.sync.dma_start(zst[66:76, :], ycols[:, 0:HALF])
                nc.scalar.dma_start(zst[76:86, :], ycols[:, HALF:NCOLS])
                for cc in (1, 0):
                    ph = slice(cc * (HALF // 2), (cc + 1) * (HALF // 2))
                    nc.gpsimd.tensor_copy(xTP[0][0:64, ph], zst[0:64, ph])
                    nc.scalar.copy(xTP[0][64:KTP, ph], zst[64:KTP, ph])
                    nc.vector.tensor_copy(xTP[1][64:KTP, ph],
                                          zst[64:KTP, ph])

            def mm(ps, lhsT, rhs, start, stop):
                nc.tensor.matmul(ps, lhsT, rhs, start=start, stop=stop)

            # Unit schedule: H u = head source cols [1024u, 1024u+1024);
            # T k = packed tail cols [1024k, min(1024k+1024, 2560)).
            # Order chosen so each unit's cross-iteration inputs (previous
            # iteration's shrink outputs) are produced well before use.
            ORDER = [("T", 2), ("H", 0), ("H", 2), ("H", 4),
                     ("T", 0), ("H", 1), ("H", 3), ("T", 1)]

            f32c = mybir.dt.float32
            for it in range(MAX_ITER):
                cur = it % 2
                nxt = (it + 1) % 2           # also the "previous" uev buffer
                m_prev = ms[it - 1] if it > 0 else 0.0
                s0 = float(1.0 + m_prev)
                s1 = float(-m_prev)
                last = it == MAX_ITER - 1
                # At it=0 s1=0 and the previous-u buffer is uninitialized:
                # alias in1 to the current buffer (contributes s1*in1 = 0).
                hp = cur if it == 0 else nxt

                for kind, u in ORDER:
                    if kind == "H":
                        gs = slice(u * GRP, (u + 1) * GRP)
                        wh = psHp.tile([KH, GRP], f32c, tag="wh", name="wh")
                        for bq in range(GRP // BLK):
                            c0 = u * GRP + bq * BLK
                            pb = slice(bq * BLK, (bq + 1) * BLK)
                            if c0 < HALF:
                                rhs2 = xTP[cur][0:76, c0:c0 + BLK]
                                w2 = wt_r["wh2a"][:]
                            else:
                                rhs2 = xTP[cur][0:KTP,
                                                c0 - HALF:c0 - HALF + BLK]
                                w2 = wt_r["wh2b"][:]
                            if it == 0:
                                mm(wh[:, pb], w2, rhs2, True, True)
                            else:
                                mm(wh[:, pb], wt_r["wh1"][:],
                                   xH[cur][:, c0:c0 + BLK], True, False)
                                mm(wh[:, pb], w2, rhs2, False, True)
                        if not last:
                            nc.scalar.copy(uevH[cur][:, gs], wh[:])
                            nc.vector._custom_dve(
                                fused_op, out=xH[nxt][:, gs],
                                in0=uevH[cur][:, gs], in1=uevH[hp][:, gs],
                                s0=s0, s1=s1, imm2=float(lam))
                        else:
                            # final iteration: shrink straight from PSUM and
                            # stream the head output out.
                            nc.vector._custom_dve(
                                fused_op, out=xH[nxt][:, gs],
                                in0=wh[:], in1=uevH[hp][:, gs],
                                s0=s0, s1=s1, imm2=float(lam))
                            nc.gpsimd.dma_start(out[0:KH, gs],
                                                xH[nxt][:, gs].bitcast(f32c))
                    else:
                        lo = u * GRP
                        hi = min(lo + GRP, HALF)
                        w = hi - lo
                        wt = psTp.tile([2 * KT, GRP], f32c, tag="wt",
                                       name="wt")
                        for bq in range(w // BLK):
                            pc0 = lo + bq * BLK
                            pb = slice(bq * BLK, (bq + 1) * BLK)
                            pcs = slice(pc0, pc0 + BLK)
                            if it == 0:
                                mm(wt[:, pb], wt_r["wts"][:],
                                   xTP[cur][:, pcs], True, True)
                            else:
                                mm(wt[:, pb], wt_r["wts"][:],
                                   xTP[cur][:, pcs], True, False)
                                mm(wt[:, pb], wt_r["wtx0"][:],
                                   xH[cur][:, pcs], False, False)
                                mm(wt[:, pb], wt_r["wtx1"][:],
                                   xH[cur][:, HALF + pc0:HALF + pc0 + BLK],
                                   False, True)
                        nc.scalar.copy(uevT[cur][:, lo:hi], wt[:, 0:w])
                        nc.vector._custom_dve(
                            fused_op, out=xTP[nxt][0:2 * KT, lo:hi],
                            in0=uevT[cur][:, lo:hi], in1=uevT[hp][:, lo:hi],
                            s0=s0, s1=s1, imm2=float(lam))
                        if last:
                            nc.sync.dma_start(
                                out[KH:K, lo:hi],
                                xTP[nxt][0:KT, lo:hi].bitcast(f32c))
                            nc.sync.dma_start(
                                out[KH:K, HALF + lo:HALF + hi],
                                xTP[nxt][KT:2 * KT, lo:hi].bitcast(f32c))
    nc.finalize()
    return nc


def _get_program(lam, ms):
    key = (round(lam, 12), tuple(round(m, 9) for m in ms))
    if _cache.get("key") != key:
        _cache["consts_meta"] = (lam, ms)
        _cache["nc"] = _build_program()
        _cache["key"] = key
    return _cache["nc"]


# ------------------------------------------------------------------- kernel
def kernel(x, Drr, Dtheta):
    from concourse.bass_utils import run_bass_kernel_spmd

    wdict, lam, ms = _host_constants(Drr, Dtheta)
    nc = _get_program(lam, ms)

    xc = np.ascontiguousarray(
        np.transpose(x.astype(np.float32), (1, 0, 2)).reshape(T, B * P))

    in_maps = []
    for c in range(NCORES):
        m = {"ycols": np.ascontiguousarray(xc[:, c * NCOLS:(c + 1) * NCOLS])}
        m.update({k: np.ascontiguousarray(v) for k, v in wdict.items()})
        in_maps.append(m)

    res = run_bass_kernel_spmd(nc, in_maps, core_ids=list(range(NCORES)))
    _cache["last_res"] = res
    full = np.concatenate([r["out"] for r in res.results], axis=1)  # [K, B*P]
    return np.ascontiguousarray(
        full.reshape(K, B, P).transpose(1, 0, 2)).astype(np.float32)


if __name__ == "__main__":
    x = np.random.randn(B, T, P).astype(np.float32)
    Drr = np.random.rand(N_POLES).astype(np.float32)
    Dtheta = np.random.rand(N_POLES).astype(np.float32)
    o = kernel(x, Drr, Dtheta)
    print(o.shape, o.dtype)
